# revision 1
# baseline (speedup 1.0000x reference)
"""GalaxyTileDecoder on 8 Trainium2 NeuronCores.

The reference pipeline (linear decode -> zero-pad -> gate -> bilinear
grid_sample -> sum over M=2 sources) collapses algebraically: the sample
grid is a pure per-source translation, sampling the padded 53x53 image at
(y, x) = (i + 2.5 - 4*locs[...,0], j + 2.5 - 4*locs[...,1]).  Folding the
integer shift (one-hot over 6 positions per axis), the bilinear weights,
the decoder bias, the galaxy_bool gate, and the M-source sum into an
expanded feature dimension turns the whole forward into one matmul:

    out[p, :] = (sum_par z_exp[p, par, :]) @ W_exp        (K=324)

with W_exp[(a, b, f), (i, j)] = canvas9[f, a+i, b+j] the 6x6 shifted
52x52 windows of the 9 basis images (8 decoder rows + bias) in a 57x57
zero canvas, and z_exp the per-source sparse coefficients
bool * z9[f] * wy[a] * wx[b].  The host computes the tiny coefficient
expansion (~0.002% of FLOPs); the device does the 10000x324x2704 matmul.

Data parallel over the ptile axis: 1250 ptiles per core, no collectives.
"""

import math
import os

import numpy as np

P_TOTAL = 10000
M = 2
N_CORES = 8
PT = P_TOTAL // N_CORES          # ptiles per core
F = 9                            # 8 decoder features + bias
A = 6                            # y-shift positions (-2..3)
B = 6                            # x-shift positions (-2..3)
K = A * B * F                    # 324 expanded features
OUT_HW = 52
COLS = OUT_HW * OUT_HW           # 2704
HALF = COLS // 2                 # 1352
CANVAS = 57

_DT_NAME = os.environ.get("BASS_GAL_DT", "bf16")

_cache = {}


def _build_program(dt_name):
    import concourse.bass as bass  # noqa: F401  (registers engines)
    import concourse.tile as tile
    from concourse import bacc, mybir

    dt_map = {
        "bf16": mybir.dt.bfloat16,
        "f32": mybir.dt.float32,
        "f32r": mybir.dt.float32r,
    }
    DT = dt_map[dt_name]

    n_batches = math.ceil(PT / 128)
    nc = bacc.Bacc(trn_type="TRN2")
    # host-blocked layouts so every DMA reads a fully contiguous DRAM block
    zt = nc.dram_tensor("zt", [K, n_batches * 128], DT, kind="ExternalInput")
    wx = nc.dram_tensor("wx", [K, COLS], DT, kind="ExternalInput")
    out = nc.dram_tensor("out", [PT, COLS], mybir.dt.float32, kind="ExternalOutput")

    KCH = [(0, 128), (128, 256), (256, K)]
    # output split into 2-bank PSUM pieces; segs within a piece are <=512
    PIECES = [(0, 1024), (1024, 2048), (2048, COLS)]
    SEGS = {0: [(0, 512), (512, 1024)],
            1: [(1024, 1536), (1536, 2048)],
            2: [(2048, 2560), (2560, COLS)]}

    with tile.TileContext(nc) as tc:
        with (
            tc.tile_pool(name="w", bufs=1) as wpool,
            tc.tile_pool(name="o", bufs=4) as opool,
            tc.tile_pool(name="ps", bufs=3, space="PSUM") as pspool,
            tc.tile_pool(name="wm", bufs=1, space="PSUM") as wmpool,
        ):
            # PE warmup: dummy matmuls spanning the input-load phase so the
            # HAM clock-gate is at 2.4 GHz when the real matmuls start.
            warm = wpool.tile([128, 128], mybir.dt.bfloat16, tag="warm")
            nc.vector.memset(warm[:], 0.0)
            wps = wmpool.tile([128, 64], mybir.dt.float32, tag="warmps")
            for _ in range(50):
                nc.tensor.matmul(wps[:, :], warm[:, 0:128], warm[:, 0:64],
                                 start=True, stop=True)
            # All inputs preloaded upfront on the HWDGE (sync) queue, in the
            # order the first batch consumes them, so the first matmul can
            # start as soon as (w piece0, z) land.
            w_tiles = {}
            z_full = []
            for ci, (k0, k1) in enumerate(KCH):
                p0, p1 = PIECES[0]
                wt = wpool.tile([k1 - k0, p1 - p0], DT, tag=f"w{ci}_0")
                nc.sync.dma_start(wt[:], wx[k0:k1, p0:p1])
                w_tiles[ci, 0] = wt
                zb = wpool.tile([k1 - k0, n_batches * 128], DT, tag=f"z{ci}")
                nc.sync.dma_start(zb[:], zt[k0:k1, :])
                z_full.append(zb)
            for pi in range(1, len(PIECES)):
                p0, p1 = PIECES[pi]
                for ci, (k0, k1) in enumerate(KCH):
                    wt = wpool.tile([k1 - k0, p1 - p0], DT, tag=f"w{ci}_{pi}")
                    nc.sync.dma_start(wt[:], wx[k0:k1, p0:p1])
                    w_tiles[ci, pi] = wt

            for bi in range(n_batches):
                b0 = bi * 128
                bs = min(128, PT - b0)
                z_b = [z_full[ci][:, bi * 128:bi * 128 + bs] for ci in range(3)]
                for pi, (p0, p1) in enumerate(PIECES):
                    pw = p1 - p0
                    ps = pspool.tile([128, 1024], mybir.dt.float32, tag="ps")
                    for ci in range(len(KCH)):
                        for (s0, s1) in SEGS[pi]:
                            nc.tensor.matmul(
                                ps[0:bs, s0 - p0:s1 - p0],
                                z_b[ci][:, 0:bs],
                                w_tiles[ci, pi][:, s0 - p0:s1 - p0],
                                start=(ci == 0),
                                stop=(ci == len(KCH) - 1),
                            )
                    osb = opool.tile([128, 1024], mybir.dt.float32, tag="osb")
                    nc.vector.tensor_copy(osb[0:bs, 0:pw], ps[0:bs, 0:pw])
                    nc.sync.dma_start(out[b0:b0 + bs, p0:p1], osb[0:bs, 0:pw])
    nc.compile()
    return nc


def _get_program(dt_name):
    if dt_name not in _cache:
        _cache[dt_name] = _build_program(dt_name)
    return _cache[dt_name]


def _host_expand(locs, galaxy_params, galaxy_bool, W_dec, b_dec, np_dtype):
    """Build zt (K, P_TOTAL) parity-summed coefficients and Wexp (K, COLS)."""
    locs = np.asarray(locs, np.float32).reshape(-1, 2)
    params = np.asarray(galaxy_params, np.float32).reshape(-1, 8)
    gbool = np.asarray(galaxy_bool, np.float32).reshape(-1, 1)
    W = np.asarray(W_dec, np.float32)
    b = np.asarray(b_dec, np.float32)
    N = locs.shape[0]

    sy = 2.5 - 4.0 * locs[:, 0]
    sx = 2.5 - 4.0 * locs[:, 1]
    m = np.floor(sy)
    k = np.floor(sx)
    fy = (sy - m).astype(np.float32)
    fx = (sx - k).astype(np.float32)
    m = m.astype(np.int64)
    k = k.astype(np.int64)
    ar = np.arange(N)
    cy = np.zeros((N, A), np.float32)
    cx = np.zeros((N, B), np.float32)
    cy[ar, m + 2] = 1.0 - fy
    cy[ar, m + 3] = fy
    cx[ar, k + 2] = 1.0 - fx
    cx[ar, k + 3] = fx

    z9 = np.concatenate([params, np.ones((N, 1), np.float32)], axis=1) * gbool
    z_exp = (cy[:, :, None, None] * cx[:, None, :, None] * z9[:, None, None, :])
    # sum the M=2 sources of each ptile (matmul is linear in z_exp)
    z_sum = z_exp.reshape(P_TOTAL, M, K).sum(axis=1)
    # per core (K, n_batches*128), zero-padded past PT
    n_batches = math.ceil(PT / 128)
    z_blk = np.zeros((N_CORES, K, n_batches * 128), np_dtype)
    zc = z_sum.astype(np_dtype).T.reshape(K, N_CORES, PT)     # (K, core, pt)
    for c in range(N_CORES):
        z_blk[c, :, 0:PT] = zc[:, c, :]

    canvas9 = np.zeros((F, CANVAS, CANVAS), np.float32)
    canvas9[:8, 3:54, 3:54] = W.reshape(8, 51, 51)
    canvas9[8, 3:54, 3:54] = b.reshape(51, 51)
    sw = np.lib.stride_tricks.sliding_window_view(canvas9, (OUT_HW, OUT_HW), axis=(1, 2))
    Wexp = np.ascontiguousarray(
        sw.transpose(1, 2, 0, 3, 4).reshape(K, COLS), dtype=np_dtype)
    return z_blk, Wexp


def kernel(locs, galaxy_params, galaxy_bool, W_dec, b_dec, _trace=False):
    import ml_dtypes
    from concourse.bass_utils import run_bass_kernel_spmd

    np_dtype = {
        "bf16": ml_dtypes.bfloat16,
        "f32": np.float32,
        "f32r": np.float32,
    }[_DT_NAME]

    z_blk, Wexp = _host_expand(
        locs, galaxy_params, galaxy_bool, W_dec, b_dec, np_dtype)

    nc = _get_program(_DT_NAME)
    in_maps = [
        {
            "zt": z_blk[c],
            "wx": Wexp,
        }
        for c in range(N_CORES)
    ]
    kwargs = {}
    if _trace:
        kwargs["trace"] = True
    res = run_bass_kernel_spmd(nc, in_maps, core_ids=list(range(N_CORES)), **kwargs)

    out = np.concatenate([res.results[c]["out"] for c in range(N_CORES)], axis=0)
    out = out.reshape(P_TOTAL, 1, OUT_HW, OUT_HW)
    if _trace:
        kernel._last_result = res
    return out, out



# revision 2
# speedup vs baseline: 1.2818x; 1.2818x over previous
"""GalaxyTileDecoder on 8 Trainium2 NeuronCores.

The reference pipeline (linear decode -> zero-pad -> gate -> bilinear
grid_sample -> sum over M=2 sources) collapses algebraically: the sample
grid is a pure per-source translation, sampling the padded 53x53 image at
(y, x) = (i + 2.5 - 4*locs[...,0], j + 2.5 - 4*locs[...,1]).  Folding the
integer shift (one-hot over 6 positions per axis), the bilinear weights,
the decoder bias, the galaxy_bool gate, and the M-source sum into an
expanded feature dimension turns the whole forward into one matmul:

    out[p, :] = z_exp[p, :] @ W_exp          (K = 6*6*9 = 324)

with W_exp[(a, b, f), (i, j)] = canvas9[f, a+i, b+j] the 6x6 shifted
52x52 windows of the 9 basis images (8 decoder rows + bias) in a 57x57
zero canvas, and z_exp the bool * z9[f] * wy[a] * wx[b] coefficients
summed over the M=2 sources.

Two refinements over the plain 3-chunk K=324 matmul:

1. Each source only touches two adjacent y-shift slots {m+2, m+3}, so
   its K-rows live in one 108-row chunk (a-pair) of W_exp when K is
   ordered (a, b, f): chunk0 = a in {0,1} (m=-2), chunk1 = a in {2,3}
   (m in {-1..1} partially), chunk2 = a in {4,5} (m=2).  Sorting ptiles
   by which chunks their two sources need yields batches that stream
   only 2 of the 3 chunks (~60% of them), cutting PE streaming from 30
   to ~24 column passes.  The batch structure is computed from the data
   at runtime and the program is compiled (and cached) per structure.

2. The output is written to DRAM in bf16 (~0.2% rounding, gate is 2e-2)
   and upcast on the host, halving the dominant write traffic.

Data parallel over ptiles: 10 batches of 128 per core, no collectives.
"""

import math
import os

import numpy as np

P_TOTAL = 10000
M = 2
N_CORES = 8
F = 9                            # 8 decoder features + bias
A = 6                            # y-shift positions
B = 6                            # x-shift positions
K = A * B * F                    # 324 expanded features
CH_ROWS = 2 * B * F              # 108 rows per a-pair chunk
OUT_HW = 52
COLS = OUT_HW * OUT_HW           # 2704
CANVAS = 57
BATCH = 128
UNIT = BATCH * N_CORES           # rows consumed by one batch slot across cores

_DT_NAME = os.environ.get("BASS_GAL_DT", "bf16")

_cache = {}

PIECES = [(0, 1024), (1024, 2048), (2048, COLS)]
SEGS = {0: [(0, 512), (512, 1024)],
        1: [(1024, 1536), (1536, 2048)],
        2: [(2048, 2560), (2560, COLS)]}


def _build_program(dt_name, struct):
    """struct: tuple of chunk-tuples, one per batch, e.g. ((0,1),(1,2),(0,1,2),...)."""
    import concourse.bass as bass  # noqa: F401  (registers engines)
    import concourse.tile as tile
    from concourse import bacc, mybir

    dt_map = {
        "bf16": mybir.dt.bfloat16,
        "f32": mybir.dt.float32,
    }
    DT = dt_map[dt_name]

    n_batches = len(struct)
    n_pass = sum(len(ch) for ch in struct)
    # flat pass -> zt column offset
    pass_col = []
    off = 0
    for ch_list in struct:
        cols = []
        for _ in ch_list:
            cols.append(off)
            off += BATCH
        pass_col.append(cols)

    nc = bacc.Bacc(trn_type="TRN2")
    zt = nc.dram_tensor("zt", [CH_ROWS, n_pass * BATCH], DT, kind="ExternalInput")
    wx = nc.dram_tensor("wx", [K, COLS], DT, kind="ExternalInput")
    out = nc.dram_tensor("out", [n_batches * BATCH, COLS], mybir.dt.bfloat16,
                         kind="ExternalOutput")

    # chunk order by first use; z split so the first batches' coefficients land early
    chunk_order = []
    for ch_list in struct:
        for ch in ch_list:
            if ch not in chunk_order:
                chunk_order.append(ch)
    z_split = pass_col[min(2, n_batches - 1)][-1] + BATCH  # cols for batches 0..2

    with tile.TileContext(nc) as tc:
        with (
            tc.tile_pool(name="w", bufs=1) as wpool,
            tc.tile_pool(name="o", bufs=3) as opool,
            tc.tile_pool(name="ps", bufs=3, space="PSUM") as pspool,
            tc.tile_pool(name="wm", bufs=1, space="PSUM") as wmpool,
        ):
            # PE warmup: dummy matmuls spanning the input-load phase so the
            # HAM clock-gate is at 2.4 GHz when the real matmuls start.
            warm = wpool.tile([128, 128], mybir.dt.bfloat16, tag="warm")
            nc.vector.memset(warm[:], 0.0)
            wps = wmpool.tile([128, 128], mybir.dt.float32, tag="warmps")
            for _ in range(30):
                nc.tensor.matmul(wps[:, :], warm[:, 0:128], warm[:, 0:128],
                                 start=True, stop=True)

            # inputs on the sync HWDGE ring, ordered so batch 0 starts earliest
            w_tiles = {}
            ch0 = chunk_order[0]
            wt = wpool.tile([CH_ROWS, COLS], DT, tag=f"w{ch0}")
            nc.sync.dma_start(wt[:], wx[ch0 * CH_ROWS:(ch0 + 1) * CH_ROWS, :])
            w_tiles[ch0] = wt

            z0 = wpool.tile([CH_ROWS, z_split], DT, tag="z0")
            nc.sync.dma_start(z0[:], zt[:, 0:z_split])
            z1 = None
            if n_pass * BATCH > z_split:
                z1 = wpool.tile([CH_ROWS, n_pass * BATCH - z_split], DT, tag="z1")

            for ci, ch in enumerate(chunk_order[1:], start=1):
                wt = wpool.tile([CH_ROWS, COLS], DT, tag=f"w{ch}")
                nc.sync.dma_start(wt[:], wx[ch * CH_ROWS:(ch + 1) * CH_ROWS, :])
                w_tiles[ch] = wt
                if ci == 1 and z1 is not None:
                    nc.sync.dma_start(z1[:], zt[:, z_split:])
            if len(chunk_order) == 1 and z1 is not None:
                nc.sync.dma_start(z1[:], zt[:, z_split:])

            def z_slice(col):
                if col < z_split:
                    return z0[:, col:col + BATCH]
                return z1[:, col - z_split:col - z_split + BATCH]

            for bi, ch_list in enumerate(struct):
                osb = opool.tile([128, COLS], mybir.dt.bfloat16, tag="osb")
                for pi, (p0, p1) in enumerate(PIECES):
                    pw = p1 - p0
                    ps = pspool.tile([128, 1024], mybir.dt.float32, tag="ps")
                    for ji, ch in enumerate(ch_list):
                        zsl = z_slice(pass_col[bi][ji])
                        for (s0, s1) in SEGS[pi]:
                            nc.tensor.matmul(
                                ps[0:128, s0 - p0:s1 - p0],
                                zsl,
                                w_tiles[ch][:, s0:s1],
                                start=(ji == 0),
                                stop=(ji == len(ch_list) - 1),
                            )
                    if pi < 2:
                        nc.vector.tensor_copy(osb[:, p0:p1], ps[0:128, 0:pw])
                    else:
                        nc.scalar.copy(osb[:, p0:p1], ps[0:128, 0:pw])
                nc.scalar.dma_start(out[bi * BATCH:(bi + 1) * BATCH, :], osb[:])
    nc.compile()
    return nc


def _get_program(dt_name, struct):
    key = (dt_name, struct)
    if key not in _cache:
        _cache[key] = _build_program(dt_name, struct)
    return _cache[key]


def _plan_batches(csets):
    """csets: (P,) uint8 bitmask of needed chunks (bit c = chunk c).
    Returns (struct, batch_ids) where batch_ids is (n_batches, N_CORES, BATCH)
    int32 of ptile ids (-1 = padding)."""
    P = csets.shape[0]
    ids = np.arange(P)
    need0 = ids[(csets & ~np.uint8(3)) == 0]          # subset of {0,1}
    need0 = need0[(csets[need0] & 1) != 0]            # actually uses chunk 0
    flex = ids[csets == 2]                            # only chunk 1
    need2 = ids[(csets & ~np.uint8(6)) == 0]
    need2 = need2[(csets[need2] & 4) != 0]
    used = np.zeros(P, bool)

    def take(arr, n):
        sel = arr[:n]
        return sel, arr[n:]

    a_rows = []
    b_rows = []
    nA = len(need0) // UNIT
    a_rows.append(need0[:nA * UNIT])
    left0 = need0[nA * UNIT:]
    if len(left0) > 0 and len(flex) >= UNIT - len(left0):
        fill, flex = take(flex, UNIT - len(left0))
        a_rows.extend([left0, fill])
        left0 = left0[:0]
        nA += 1
    nB = len(need2) // UNIT
    b_rows.append(need2[:nB * UNIT])
    left2 = need2[nB * UNIT:]
    if len(left2) > 0 and len(flex) >= UNIT - len(left2):
        fill, flex = take(flex, UNIT - len(left2))
        b_rows.extend([left2, fill])
        left2 = left2[:0]
        nB += 1
    a_ids = np.concatenate(a_rows) if a_rows else np.empty(0, np.int64)
    b_ids = np.concatenate(b_rows) if b_rows else np.empty(0, np.int64)
    used[a_ids] = True
    used[b_ids] = True
    c_ids = ids[~used]
    nC = math.ceil(len(c_ids) / UNIT) if len(c_ids) else 0
    c_ids = np.concatenate([c_ids, np.full(nC * UNIT - len(c_ids), -1, np.int64)])

    struct = ((0, 1),) * nA + ((1, 2),) * nB + ((0, 1, 2),) * nC
    all_ids = np.concatenate([a_ids, b_ids, c_ids]).astype(np.int32)
    batch_ids = all_ids.reshape(len(struct), N_CORES, BATCH)
    return struct, batch_ids


def _host_expand(locs, galaxy_params, galaxy_bool, W_dec, b_dec, np_dtype):
    """Coefficients z_sum (P, K), chunk-need bitmask (P,), W_exp (K, COLS)."""
    locs = np.asarray(locs, np.float32).reshape(-1, 2)
    params = np.asarray(galaxy_params, np.float32).reshape(-1, 8)
    gbool = np.asarray(galaxy_bool, np.float32).reshape(-1, 1)
    W = np.asarray(W_dec, np.float32)
    b = np.asarray(b_dec, np.float32)
    N = locs.shape[0]
    P = N // M

    sy = 2.5 - 4.0 * locs[:, 0]
    sx = 2.5 - 4.0 * locs[:, 1]
    m = np.clip(np.floor(sy), -2, 2)
    k = np.clip(np.floor(sx), -2, 2)
    fy = (sy - m).astype(np.float32)
    fx = (sx - k).astype(np.float32)
    m = m.astype(np.int64)
    k = k.astype(np.int64)
    ar = np.arange(N)
    cy = np.zeros((N, A), np.float32)
    cx = np.zeros((N, B), np.float32)
    cy[ar, m + 2] = 1.0 - fy
    cy[ar, m + 3] = fy
    cx[ar, k + 2] = 1.0 - fx
    cx[ar, k + 3] = fx

    z9 = np.concatenate([params, np.ones((N, 1), np.float32)], axis=1) * gbool
    z_exp = (cy[:, :, None, None] * cx[:, None, :, None] * z9[:, None, None, :])
    z_sum = z_exp.reshape(P, M, K).sum(axis=1).astype(np_dtype)

    # chunk bitmask per source: slots {m+2, m+3} -> a-pair chunks
    c_lo = (m + 2) // 2
    c_hi = (m + 3) // 2
    smask = (1 << c_lo.astype(np.uint8)) | (1 << c_hi.astype(np.uint8))
    csets = (smask.reshape(P, M)[:, 0] | smask.reshape(P, M)[:, 1]).astype(np.uint8)

    canvas9 = np.zeros((F, CANVAS, CANVAS), np.float32)
    canvas9[:8, 3:54, 3:54] = W.reshape(8, 51, 51)
    canvas9[8, 3:54, 3:54] = b.reshape(51, 51)
    sw = np.lib.stride_tricks.sliding_window_view(canvas9, (OUT_HW, OUT_HW), axis=(1, 2))
    Wexp = np.ascontiguousarray(
        sw.transpose(1, 2, 0, 3, 4).reshape(K, COLS), dtype=np_dtype)
    return z_sum, csets, Wexp


def kernel(locs, galaxy_params, galaxy_bool, W_dec, b_dec, _trace=False):
    import ml_dtypes
    from concourse.bass_utils import run_bass_kernel_spmd

    np_dtype = {
        "bf16": ml_dtypes.bfloat16,
        "f32": np.float32,
    }[_DT_NAME]

    z_sum, csets, Wexp = _host_expand(
        locs, galaxy_params, galaxy_bool, W_dec, b_dec, np_dtype)
    struct, batch_ids = _plan_batches(csets)
    n_batches = len(struct)
    n_pass = sum(len(ch) for ch in struct)

    # per-core zt: [108, n_pass*128], one 128-col block per (batch, chunk) pass
    z_pad = np.concatenate([z_sum, np.zeros((1, K), np_dtype)], axis=0)  # -1 -> 0
    zt = np.empty((N_CORES, CH_ROWS, n_pass * BATCH), np_dtype)
    for c in range(N_CORES):
        off = 0
        for bi, ch_list in enumerate(struct):
            rows = batch_ids[bi, c]
            zb = z_pad[rows]                       # (128, K)
            for ch in ch_list:
                zt[c, :, off:off + BATCH] = zb[:, ch * CH_ROWS:(ch + 1) * CH_ROWS].T
                off += BATCH

    nc = _get_program(_DT_NAME, struct)
    in_maps = [{"zt": zt[c], "wx": Wexp} for c in range(N_CORES)]
    kwargs = {}
    if _trace:
        kwargs["trace"] = True
    res = run_bass_kernel_spmd(nc, in_maps, core_ids=list(range(N_CORES)), **kwargs)

    P = z_sum.shape[0]
    full = np.empty((P + 1, COLS), np.float32)
    safe_ids = np.where(batch_ids < 0, P, batch_ids)   # (n_batches, N_CORES, 128)
    for c in range(N_CORES):
        co = np.asarray(res.results[c]["out"]).astype(np.float32)
        full[safe_ids[:, c, :].reshape(-1)] = co.reshape(n_batches * BATCH, COLS)
    out = full[:P].reshape(P, 1, OUT_HW, OUT_HW)
    if _trace:
        kernel._last_result = res
    return out, out
